# revision 1
# baseline (speedup 1.0000x reference)
"""Bahdanau-style attention kernel for Trainium2, data-parallel over batch.

Math (per (s, b)):
    pre[s,b,:]  = We @ enc[s,b,:] + Wh @ hidden[b,:] + attn_b      (H outputs)
    energies    = score_w . tanh(pre)                               -> [S, B]
    out         = softmax over S of (energies masked to -1e12)      -> [B, 1, S]

Sharding: B=16 batches split 2-per-core over 8 NeuronCores; weights are
replicated; no collectives. Each core runs one identical Bass program on
its own input slice.

Per-core pipeline (s-on-partitions layout):
  - Main GEMM: out tiles are pre[s=128, h_out=512] in PSUM, with enc^T
    chunks [h_in=128, s=128] stationary and We^T chunks [h_in=128,
    h_out=512] moving, in fp32r (tf32: full PE rate at free dim >=256,
    ~1e-4 end-to-end error vs the fp32 reference).
  - bias (attn_b + Wh@hidden, per-batch constant over s) is broadcast to
    all 128 partitions once per batch via gpsimd partition_broadcast,
    then added on VectorE (scalar_tensor_tensor) while reading PSUM.
  - tanh on ScalarE; the score contraction over h_out runs on VectorE as
    scalar_tensor_tensor(bypass, mult) with accum_out row-sums (NOTE:
    tensor_tensor_reduce hangs TRN2 hardware here - do not use it), so
    the PE runs only the 512 main matmuls.
  - energies land as [s=128, 16]; softmax is lane-parallel: mask via
    scalar_tensor_tensor, 32x32 DVE transposes for the cross-partition
    sum and the 1/sum broadcast, exp+sum fused via activation accum_out.
    Max-subtraction is skipped (energies are O(1) bounded by |score_w|_1
    so exp cannot overflow, and exp(-1e12) == 0 exactly, matching the
    reference's masked softmax).
  - HAM discipline (measured): weights load fully before compute; ~40
    never-read warm-up matmuls keep the PE at full clock through the DMA
    prologue; the first enc group's DMAs are emitted k-reversed so the
    real stream starts dense instead of dribbling (dribble re-throttles
    the PE to half clock for ~20 us).
"""

import sys

for _p in ("/opt/trn_rl_repo", "/opt/pypackages"):
    if _p not in sys.path:
        sys.path.append(_p)

import numpy as np

from concourse import bacc, mybir, tile
from concourse.bass_utils import run_bass_kernel_spmd

H = 1024
S = 2048
B = 16
NCORES = 8
BL = B // NCORES  # local batches per core
P = 128
KT = H // P  # h_in tiles
NF = 512  # h_out half width (moving free dim)
SH = 2  # s halves (1024 each) per enc DMA group
ST = (S // SH) // P  # s-tiles of 128 per half = 8
NT = S // P  # energies columns per batch = 16

F32 = mybir.dt.float32
F32R = mybir.dt.float32r
I32 = mybir.dt.int32
AF = mybir.ActivationFunctionType
AX = mybir.AxisListType
OP = mybir.AluOpType


def _build_program():
    nc = bacc.Bacc("TRN2", target_bir_lowering=False, debug=False, num_devices=NCORES)

    encT = nc.dram_tensor("encT", [BL, H, S], F32R, kind="ExternalInput").ap()
    weT = nc.dram_tensor("weT", [H, H], F32R, kind="ExternalInput").ap()
    whT = nc.dram_tensor("whT", [H, H], F32R, kind="ExternalInput").ap()
    hid16 = nc.dram_tensor("hid16", [P, KT * BL], F32R, kind="ExternalInput").ap()
    battn = nc.dram_tensor("battn", [1, H], F32, kind="ExternalInput").ap()
    score_row = nc.dram_tensor("score_row", [1, H], F32, kind="ExternalInput").ap()
    maskT = nc.dram_tensor("maskT", [BL, P, NT], I32, kind="ExternalInput").ap()
    out = nc.dram_tensor("out", [BL, S], F32, kind="ExternalOutput").ap()

    with tile.TileContext(nc) as tc:
        with (
            tc.tile_pool(name="consts", bufs=1) as cpool,
            tc.tile_pool(name="weights", bufs=1) as wpool,
            tc.tile_pool(name="enc", bufs=2) as epool,
            tc.tile_pool(name="work", bufs=4) as ppool,
            tc.tile_pool(name="soft", bufs=1) as spool,
            tc.tile_pool(name="mm", bufs=5, space="PSUM") as mmpool,
            tc.tile_pool(name="aux", bufs=1, space="PSUM") as auxpool,
        ):
            # --- tiny constants on the SWDGE queue (their ~2us fixed costs
            # must not serialize ahead of the weight stream on sync) -------
            hid_sb = cpool.tile([P, KT * BL], F32R, tag="hid")
            nc.gpsimd.dma_start(hid_sb[:], hid16[:])
            battn_sb = cpool.tile([1, H], F32, tag="battn")
            nc.gpsimd.dma_start(battn_sb[:], battn[:])
            score_sb = cpool.tile([1, H], F32, tag="score_row")
            nc.gpsimd.dma_start(score_sb[:], score_row[:])
            mask_sb = []
            for b in range(BL):
                m = cpool.tile([P, NT], I32, tag=f"maskT{b}", name=f"maskT{b}")
                nc.gpsimd.dma_start(m[:], maskT[b])
                mask_sb.append(m)
            # ones row for the 1/sum spread in the softmax tail
            ones_f = cpool.tile([1, P], F32, tag="ones_f")
            nc.vector.memset(ones_f[:], 1.0)

            # --- weights first: a dense k-loop needs all of We before any
            # psum group completes; interleaving enc here was measured
            # (twice) to DMA-starve the PE into HAM half-clock ------------
            we_sb = []
            for k in range(KT):
                t = wpool.tile([P, H], F32R, tag=f"we{k}")
                nc.sync.dma_start(t[:], weT[k * P : (k + 1) * P, :])
                we_sb.append(t)

            # --- PE warm-up filler ----------------------------------------
            # The weight/enc prologue DMA would leave the PE idle, so HAM
            # holds it at half clock for ~3.4 us of sustained work after
            # the real matmuls begin (~12 us measured penalty). Chew
            # through never-read matmuls on the first weight tile instead.
            junk_ps = auxpool.tile([P, NF], F32, tag="junk")
            for i in range(46):
                nc.tensor.matmul(
                    junk_ps[:],
                    lhsT=we_sb[0][:, 0:P],
                    rhs=we_sb[0][:, 0:NF],
                    start=True,
                    stop=True,
                    skip_group_check=True,
                )

            wh_sb = []
            for k in range(KT):
                t = wpool.tile([P, H], F32R, tag=f"wh{k}")
                nc.sync.dma_start(t[:], whT[k * P : (k + 1) * P, :])
                wh_sb.append(t)

            # --- hidden projection, row layout: hidp[b] is [1, H] on
            # partition 0; bias_row[b] = attn_b + hidp[b] -------------------
            bias_row = []
            for b in range(BL):
                r = cpool.tile([1, H], F32, tag=f"bias_row{b}", name=f"bias_row{b}")
                bias_row.append(r)
            for b in range(BL):
                for hh in range(2):
                    ps = auxpool.tile(
                        [1, NF], F32, tag="hidp", bufs=2, name=f"hidp{b}{hh}"
                    )
                    for k in range(KT):
                        nc.tensor.matmul(
                            ps[:],
                            lhsT=hid_sb[:, k * BL + b : k * BL + b + 1],
                            rhs=wh_sb[k][:, hh * NF : (hh + 1) * NF],
                            start=(k == 0),
                            stop=(k == KT - 1),
                        )
                    nc.vector.tensor_add(
                        bias_row[b][0:1, hh * NF : (hh + 1) * NF],
                        ps[:],
                        battn_sb[0:1, hh * NF : (hh + 1) * NF],
                    )

            # --- broadcast bias/score rows to all 128 partitions (bias
            # first: the first stt blocks on it) ---------------------------
            bias_bc = []
            for b in range(BL):
                t = cpool.tile([P, H], F32, tag=f"bias_bc{b}", name=f"bias_bc{b}")
                nc.gpsimd.partition_broadcast(t[:], bias_row[b][:])
                bias_bc.append(t)
            score_bc = cpool.tile([P, H], F32, tag="score_bc")
            nc.gpsimd.partition_broadcast(score_bc[:], score_sb[:])

            # --- second warm-up block: the first enc group is still ~5 us
            # out at this point; keep the PE dense until it lands ----------
            for i in range(28):
                nc.tensor.matmul(
                    junk_ps[:],
                    lhsT=wh_sb[0][:, 0:P],
                    rhs=wh_sb[0][:, 0:NF],
                    start=True,
                    stop=True,
                    skip_group_check=True,
                )

            # --- energies accumulators ------------------------------------
            energies = [
                spool.tile([P, NT], F32, tag=f"energy{b}", name=f"energy{b}")
                for b in range(BL)
            ]

            # --- main loop -------------------------------------------------
            for b in range(BL):
                for sh in range(SH):
                    # Group 0 is split into two half-width sub-groups so the
                    # DMA front-load before dense compute is 2 MB smaller,
                    # and its DMAs are emitted k-reversed so the program-
                    # order-first matmul unblocks only once a whole sub-group
                    # is resident: the PE runs dense instead of dribbling
                    # behind the DMA (dribble re-throttles HAM to half
                    # clock).
                    g0 = b == 0 and sh == 0
                    if g0:
                        halves = []
                        hw_ = S // SH // 2
                        for half in range(2):
                            em = {}
                            for k in range(KT - 1, -1, -1):
                                t = epool.tile(
                                    [P, hw_],
                                    F32R,
                                    tag=f"enc{k}",
                                    name=f"enc0_{half}_{k}",
                                )
                                nc.sync.dma_start(
                                    t[:],
                                    encT[
                                        0,
                                        k * P : (k + 1) * P,
                                        half * hw_ : (half + 1) * hw_,
                                    ],
                                )
                                em[k] = t
                            halves.append([em[k] for k in range(KT)])
                    else:
                        enc_m = {}
                        for k in range(KT):
                            t = epool.tile(
                                [P, S // SH],
                                F32R,
                                tag=f"enc{k}",
                                name=f"enc_{b}_{sh}_{k}",
                            )
                            nc.sync.dma_start(
                                t[:],
                                encT[
                                    b,
                                    k * P : (k + 1) * P,
                                    sh * (S // SH) : (sh + 1) * (S // SH),
                                ],
                            )
                            enc_m[k] = t
                        enc_t = [enc_m[k] for k in range(KT)]

                    for st in range(ST):
                        tix = sh * ST + st  # energies column
                        accs = []
                        for hh in range(2):
                            if g0:
                                src_t = halves[st // (ST // 2)]
                                soff = (st % (ST // 2)) * P
                            else:
                                src_t = enc_t
                                soff = st * P
                            ps = mmpool.tile([P, NF], F32, tag="mm")
                            for k in range(KT):
                                nc.tensor.matmul(
                                    ps[:],
                                    lhsT=src_t[k][:, soff : soff + P],
                                    rhs=we_sb[k][:, hh * NF : (hh + 1) * NF],
                                    start=(k == 0),
                                    stop=(k == KT - 1),
                                )
                            pre = ppool.tile([P, NF], F32, tag="pre")
                            nc.vector.scalar_tensor_tensor(
                                pre[:],
                                ps[:],
                                1.0,
                                bias_bc[b][:, hh * NF : (hh + 1) * NF],
                                op0=OP.mult,
                                op1=OP.add,
                            )
                            proj = ppool.tile([P, NF], F32, tag="proj")
                            nc.scalar.activation(proj[:], pre[:], AF.Tanh)
                            # score contraction on DVE: out=(proj bypass) *
                            # score, accum_out = row sums (tensor_tensor_reduce
                            # crashes TRN2 here; scalar_tensor_tensor's accum
                            # path is HW-proven)
                            scr = ppool.tile([P, NF], F32, tag="scr")
                            acc = ppool.tile(
                                [P, 1], F32, tag=f"eacc{hh}", name=f"eacc_{b}_{tix}_{hh}"
                            )
                            accs.append(acc)
                            nc.vector.scalar_tensor_tensor(
                                scr[:],
                                proj[:],
                                0.0,
                                score_bc[:, hh * NF : (hh + 1) * NF],
                                op0=OP.bypass,
                                op1=OP.mult,
                                accum_out=acc[:],
                            )
                        nc.vector.tensor_add(
                            energies[b][:, tix : tix + 1], accs[0][:], accs[1][:]
                        )

            # --- lane-parallel masked softmax + store ---------------------
            for b in range(BL):
                # masked = energies + mask * -1e12, in a [128, 32] tile so
                # the 32x32 DVE transpose can run (pad columns zeroed)
                masked = spool.tile([P, 32], F32, tag=f"masked{b}", name=f"masked{b}")
                nc.vector.memset(masked[:, NT:32], 0.0)
                nc.vector.scalar_tensor_tensor(
                    masked[:, 0:NT],
                    mask_sb[b][:],
                    -1.0e12,
                    energies[b][:],
                    op0=OP.mult,
                    op1=OP.add,
                )
                maskedT = spool.tile(
                    [32, P], F32, tag=f"maskedT{b}", name=f"maskedT{b}"
                )
                for q in range(4):  # DVE transpose is square-only: 4x 32x32
                    nc.vector.transpose(
                        maskedT[:, q * 32 : (q + 1) * 32],
                        masked[q * 32 : (q + 1) * 32, :],
                    )
                # exp rows 0..15 with fused per-partition sums
                sq = spool.tile([32, 32], F32, tag=f"sq{b}", name=f"sq{b}")
                nc.vector.memset(sq[:], 0.0)
                expT = spool.tile([32, P], F32, tag=f"expT{b}", name=f"expT{b}")
                nc.scalar.activation(
                    expT[0:NT, :],
                    maskedT[0:NT, :],
                    AF.Exp,
                    accum_out=sq[0:NT, 0:1],
                )
                # cross-partition sum: transpose the sums column into a row
                sqT = spool.tile([32, 32], F32, tag=f"sqT{b}", name=f"sqT{b}")
                nc.vector.transpose(sqT[:], sq[:])
                tot = spool.tile([1, 1], F32, tag=f"tot{b}", name=f"tot{b}")
                nc.vector.reduce_sum(tot[:], sqT[0:1, :], axis=AX.X)
                rec = spool.tile([1, 1], F32, tag=f"rec{b}", name=f"rec{b}")
                nc.vector.reciprocal(rec[:], tot[:])
                # spread 1/sum to 16 partitions: row of rec, transpose back
                nc.vector.tensor_scalar_mul(sqT[0:1, :], ones_f[0:1, 0:32], rec[:])
                nc.vector.transpose(sq[:], sqT[:])
                outT = spool.tile([32, P], F32, tag=f"outT{b}", name=f"outT{b}")
                nc.vector.tensor_scalar_mul(
                    outT[0:NT, :], expT[0:NT, :], sq[0:NT, 0:1]
                )
                nc.sync.dma_start(
                    out[b : b + 1, :].rearrange("o (t p) -> (o t) p", p=P),
                    outT[0:NT, :],
                )

    nc.compile()
    return nc


_NC = None


def _get_program():
    global _NC
    if _NC is None:
        _NC = _build_program()
    return _NC


def make_in_maps(hidden, encoder_outputs, seq_mask, attn_w, attn_b, score_w):
    """Slice/relayout the full inputs into the 8 per-core input maps."""
    hidden = np.asarray(hidden, dtype=np.float32)
    encoder_outputs = np.asarray(encoder_outputs, dtype=np.float32)
    seq_mask = np.ascontiguousarray(np.asarray(seq_mask, dtype=np.int32))
    attn_w = np.asarray(attn_w, dtype=np.float32)
    attn_b = np.asarray(attn_b, dtype=np.float32)
    score_w = np.asarray(score_w, dtype=np.float32)

    weT = np.ascontiguousarray(attn_w[:, H:].T)  # [h_in, h_out]
    whT = np.ascontiguousarray(attn_w[:, :H].T)  # [h_in, h_out]
    battn = np.ascontiguousarray(attn_b[None, :])  # [1, H]
    score_row = np.ascontiguousarray(score_w)  # [1, H]
    encT = encoder_outputs.transpose(1, 2, 0)  # [B, H, S]
    hidT = hidden[0].T  # [H, B]
    # maskT[b, p, t] = seq_mask[b, t*P + p]
    maskT = np.ascontiguousarray(
        seq_mask.reshape(B, NT, P).transpose(0, 2, 1)
    )

    in_maps = []
    for c in range(NCORES):
        bsl = slice(c * BL, (c + 1) * BL)
        hid16 = np.ascontiguousarray(
            hidT[:, bsl].reshape(KT, P, BL).transpose(1, 0, 2).reshape(P, KT * BL)
        )
        in_maps.append(
            {
                "encT": np.ascontiguousarray(encT[bsl]),
                "weT": weT,
                "whT": whT,
                "hid16": hid16,
                "battn": battn,
                "score_row": score_row,
                "maskT": np.ascontiguousarray(maskT[bsl]),
            }
        )
    return in_maps


def gather_output(results):
    outs = np.concatenate([results[c]["out"] for c in range(NCORES)], axis=0)
    return np.ascontiguousarray(outs[:, None, :].astype(np.float32))


def kernel(hidden, encoder_outputs, seq_mask, attn_w, attn_b, score_w):
    nc = _get_program()
    in_maps = make_in_maps(
        hidden, encoder_outputs, seq_mask, attn_w, attn_b, score_w
    )
    last_err = None
    for _attempt in range(3):
        try:
            res = run_bass_kernel_spmd(nc, in_maps, list(range(NCORES)))
            return gather_output(res.results)
        except Exception as e:  # rare transient NRT device errors on first exec
            last_err = e
            import time as _time

            _time.sleep(2.0)
    raise last_err

